# revision 5
# baseline (speedup 1.0000x reference)
"""Trainium2 Bass kernel for nn_ConvLayer_17918603559109 (gnn_message_passing).

Computes out = BatchNorm(relu(L0 @ (X@W0) + L1 @ (X@W1) + L2 @ (X@W2) + bias))
for N=8192 nodes, C=64 channels, distributed across 8 NeuronCores.

Sharding strategy (per the row-partition hint):
  - Node (output-row) dimension split 8 ways: core c owns rows [1024c, 1024(c+1)).
  - Each Lk row-shard is staged in HBM k-major (transposed on host during
    sharding) so the contraction dim lands on SBUF partitions and every DMA
    runs with >=4KB contiguous descriptors at full HBM bandwidth.
  - X (as X^T), Wk, bias/gamma/beta replicated on every core.
  - Each core computes Y_c^T = sum_k (X@Wk chunks)^T-stationary @ Lk^T
    accumulated over 64 contraction chunks in PSUM (float32r matmuls:
    TF32-rate streaming, fp32 PSUM accumulate).
  - BatchNorm stats need global per-channel sums: each core computes partial
    [sum, sum_sq] over its 1024 rows, then a tiny 512B AllReduce across the
    8 cores; normalization is applied locally; output shard is Y_c^T.
"""

import sys

if "/opt/trn_rl_repo" not in sys.path:
    sys.path.insert(0, "/opt/trn_rl_repo")

import numpy as np

N_CORES = 8
N = 8192
C = 64
ROWS = N // N_CORES          # 1024 output rows per core
KCH = N // 128               # 64 contraction chunks of 128
BN_EPS = 1e-5
MM_F = 512                   # moving free-dim per matmul (fp32 PSUM-bank limit)

_cache: dict = {}


def _build(num_devices=N_CORES, with_collective=True):
    import concourse.bacc as bacc
    import concourse.mybir as mybir
    import concourse.tile as tile

    F32 = mybir.dt.float32
    F32R = mybir.dt.float32r
    AF = mybir.ActivationFunctionType
    ALU = mybir.AluOpType

    nc = bacc.Bacc("TRN2", target_bir_lowering=False, debug=False,
                   enable_asserts=False, num_devices=num_devices)

    lt_d = [nc.dram_tensor(f"lt{k}", [N, ROWS], F32, kind="ExternalInput")
            for k in range(3)]
    xt_d = nc.dram_tensor("xt", [C, N], F32, kind="ExternalInput")
    w_d = nc.dram_tensor("w", [C, 3 * C], F32, kind="ExternalInput")
    bgb_d = nc.dram_tensor("bgb", [C, 3], F32, kind="ExternalInput")
    out_d = nc.dram_tensor("out", [C, ROWS], F32, kind="ExternalOutput")

    with tile.TileContext(nc) as tc:
        with (tc.tile_pool(name="const", bufs=1) as cpool,
              tc.tile_pool(name="zpool", bufs=KCH) as zpool,
              tc.tile_pool(name="lpool", bufs=9) as lpool,
              tc.tile_pool(name="ypsum", bufs=1, space="PSUM") as ypsum,
              tc.tile_pool(name="zpsum", bufs=2, space="PSUM") as zpsum,
              tc.tile_pool(name="epool", bufs=1) as epool,
              tc.tile_pool(name="dpool", bufs=1, space="DRAM") as dpool):

            xt_t = cpool.tile([C, N], F32R, name="xt_t")
            nc.sync.dma_start(xt_t[:], xt_d.ap()[:].bitcast(F32R))
            w_t = cpool.tile([C, 3 * C], F32R, name="w_t")
            nc.sync.dma_start(w_t[:], w_d.ap()[:].bitcast(F32R))
            bgb_t = cpool.tile([C, 3], F32, name="bgb_t")
            nc.sync.dma_start(bgb_t[:], bgb_d.ap()[:])

            # Prologue: Z chunks. Z[i] = (X @ [W0|W1|W2])[128i:128(i+1), :]
            # via stationary X^T chunk [64, 128], moving [W0|W1|W2] [64, 192].
            zts = []
            for i in range(KCH):
                zp = zpsum.tile([128, 3 * C], F32, tag="zps", name=f"zp{i}")
                nc.tensor.matmul(zp[:], xt_t[:, 128 * i:128 * (i + 1)], w_t[:],
                                 start=True, stop=True)
                zt = zpool.tile([128, 3 * C], F32R, tag="zt", name=f"zt{i}")
                nc.vector.tensor_copy(zt[:], zp[:])
                zts.append(zt)

            # Main loop: Y^T[64, 1024] += Z_k-chunk^stationary @ Lk^T-slab.
            yt_ps = ypsum.tile([C, ROWS], F32, name="yt_ps")
            n_half = ROWS // MM_F
            for i in range(KCH):
                for k in range(3):
                    ltile = lpool.tile([128, ROWS], F32R, tag="lt",
                                       name=f"lt{i}_{k}")
                    nc.sync.dma_start(
                        ltile[:],
                        lt_d[k].ap()[128 * i:128 * (i + 1), :].bitcast(F32R))
                    first = (i == 0 and k == 0)
                    last = (i == KCH - 1 and k == 2)
                    for h in range(n_half):
                        nc.tensor.matmul(
                            yt_ps[:, MM_F * h:MM_F * (h + 1)],
                            zts[i][:, C * k:C * (k + 1)],
                            ltile[:, MM_F * h:MM_F * (h + 1)],
                            start=first, stop=last)

            # Epilogue: bias+relu (+ running per-channel sum), square (+ sum).
            yt = epool.tile([C, ROWS], F32, name="yt")
            stats = epool.tile([C, 2], F32, name="stats")
            nc.scalar.activation(yt[:], yt_ps[:], AF.Relu,
                                 bias=bgb_t[:, 0:1], accum_out=stats[:, 0:1])
            sq = epool.tile([C, ROWS], F32, name="sq")
            nc.scalar.activation(sq[:], yt[:], AF.Square,
                                 accum_out=stats[:, 1:2])

            # Global BN stats: 512B AllReduce across the 8 cores.
            sg = epool.tile([C, 2], F32, name="sg")
            if with_collective:
                cin = dpool.tile([C, 2], F32, name="cin")
                cout = dpool.tile([C, 2], F32, name="cout", addr_space="Shared")
                nc.sync.dma_start(cin[:], stats[:])
                nc.gpsimd.collective_compute(
                    "AllReduce", ALU.add,
                    replica_groups=[list(range(num_devices))],
                    ins=[cin.opt()], outs=[cout.opt()])
                nc.sync.dma_start(sg[:], cout[:])
            else:
                nc.vector.tensor_copy(sg[:], stats[:])

            # scale = gamma * rsqrt(var + eps); shift = beta - mean * scale
            mean = epool.tile([C, 1], F32, name="mean")
            nc.vector.tensor_scalar_mul(mean[:], sg[:, 0:1], 1.0 / N)
            ex2 = epool.tile([C, 1], F32, name="ex2")
            nc.vector.tensor_scalar_mul(ex2[:], sg[:, 1:2], 1.0 / N)
            m2 = epool.tile([C, 1], F32, name="m2")
            nc.vector.tensor_mul(m2[:], mean[:], mean[:])
            var = epool.tile([C, 1], F32, name="var")
            nc.vector.tensor_sub(var[:], ex2[:], m2[:])
            nc.vector.tensor_scalar_add(var[:], var[:], BN_EPS)
            std = epool.tile([C, 1], F32, name="std")
            nc.scalar.activation(std[:], var[:], AF.Sqrt)
            inv = epool.tile([C, 1], F32, name="inv")
            nc.vector.reciprocal(inv[:], std[:])
            scl = epool.tile([C, 1], F32, name="scl")
            nc.vector.tensor_mul(scl[:], inv[:], bgb_t[:, 1:2])
            tmp = epool.tile([C, 1], F32, name="tmp")
            nc.vector.tensor_mul(tmp[:], mean[:], scl[:])
            shf = epool.tile([C, 1], F32, name="shf")
            nc.vector.tensor_sub(shf[:], bgb_t[:, 2:3], tmp[:])

            ot = epool.tile([C, ROWS], F32, name="ot")
            nc.vector.tensor_scalar(ot[:], yt[:], scl[:], shf[:],
                                    op0=ALU.mult, op1=ALU.add)
            nc.sync.dma_start(out_d.ap()[:], ot[:])

    nc.compile()
    return nc


def _get_nc():
    if "nc" not in _cache:
        _cache["nc"] = _build()
    return _cache["nc"]


def kernel(X, L0, L1, L2, W0, W1, W2, bias, gamma, beta):
    from concourse import bass_utils

    X = np.asarray(X, dtype=np.float32)
    Ls = [np.asarray(L, dtype=np.float32) for L in (L0, L1, L2)]
    W = np.ascontiguousarray(
        np.concatenate([np.asarray(Wk, dtype=np.float32) for Wk in (W0, W1, W2)],
                       axis=1))
    bgb = np.ascontiguousarray(
        np.stack([np.asarray(v, dtype=np.float32) for v in (bias, gamma, beta)],
                 axis=1))
    xt = np.ascontiguousarray(X.T)

    nc = _get_nc()

    in_maps = []
    for c in range(N_CORES):
        rows = slice(ROWS * c, ROWS * (c + 1))
        m = {"xt": xt, "w": W, "bgb": bgb}
        for k in range(3):
            m[f"lt{k}"] = np.ascontiguousarray(Ls[k][rows].T)
        in_maps.append(m)

    res = bass_utils.run_bass_kernel_spmd(nc, in_maps,
                                          core_ids=list(range(N_CORES)))

    out = np.empty((N, C), dtype=np.float32)
    for c in range(N_CORES):
        out[ROWS * c:ROWS * (c + 1)] = res.results[c]["out"].T
    return out
